# revision 1
# baseline (speedup 1.0000x reference)
"""CropAndResize (TF-style, crop 14x14) on 8 Trainium2 NeuronCores.

Strategy (data-parallel over ROIs, grouped by image):
  - Host: transpose image to channel-last [B, H, W, C]; group the 1000 boxes
    by box_ind so core k handles image k plus its boxes (padded to a common
    count so all 8 cores run one SPMD program).
  - Host computes the TF sampling grid (bit-exact f32 mirror of the
    reference): per output pixel the 4 bilinear corners are two ADJACENT
    column pairs (rows ti/bi, cols xs, xs+1). Each pair is 2*256 floats =
    2KB contiguous in channel-last layout.
  - Device: per chunk of boxes, one SWDGE dma_gather fetches all 2KB pairs
    (HBM -> SBUF, pixel on partitions, channels on the free dim), then the
    exact lerp runs on DVE/Pool/ACT with per-partition scalar weights:
        top = T0 + (T1-T0)*xw ; bot = B0 + (B1-B0)*xw
        val = (top + (bot-top)*yw) * valid
    and the result streams back to DRAM pixel-major.
  - Host: scatter per-core outputs back to the original box order and
    transpose to [N, C, 14, 14].
"""

import numpy as np

import concourse.bacc as bacc
import concourse.bass as bass
import concourse.tile as tile
from concourse import mybir, library_config, bass_utils

H, W, C = 100, 152, 256
CROP = 14
PX = CROP * CROP          # 196 pixels per box
P = 128                   # SBUF partitions
NCORES = 8
CH = 8                    # boxes per chunk
QPAD = ((CH * PX + P - 1) // P) * P   # padded pixels per chunk (1664)
S = QPAD // P             # output slots per chunk (13)
NI = 2 * QPAD             # gather descriptors per chunk (top+bottom pairs)
NPIX = H * W              # 15200 gatherable columns per image

F32 = mybir.dt.float32
I16 = mybir.dt.int16
MULT = mybir.AluOpType.mult
ADD = mybir.AluOpType.add
SUB = mybir.AluOpType.subtract

_cache = {}
LAST_EXEC_NS = None


def _grid_params(boxes):
    """Bit-exact f32 mirror of the reference sampling-grid math."""
    f = np.float32
    y1, x1, y2, x2 = boxes[:, 0], boxes[:, 1], boxes[:, 2], boxes[:, 3]
    h_scale = (y2 - y1) * f(H - 1) / f(CROP - 1)
    w_scale = (x2 - x1) * f(W - 1) / f(CROP - 1)
    ar = np.arange(CROP, dtype=np.float32)
    in_y = y1[:, None] * f(H - 1) + ar[None, :] * h_scale[:, None]
    in_x = x1[:, None] * f(W - 1) + ar[None, :] * w_scale[:, None]
    valid_y = (in_y >= 0) & (in_y <= H - 1)
    valid_x = (in_x >= 0) & (in_x <= W - 1)
    top = np.floor(in_y)
    left = np.floor(in_x)
    y_lerp = (in_y - top).astype(np.float32)
    x_lerp = (in_x - left).astype(np.float32)
    ti = np.clip(top, 0, H - 1).astype(np.int32)
    bi = np.clip(top + 1, 0, H - 1).astype(np.int32)
    li = np.clip(left, 0, W - 1).astype(np.int32)
    ri = np.clip(left + 1, 0, W - 1).astype(np.int32)
    # column pair start + effective in-pair x lerp
    xs = np.minimum(li, W - 2).astype(np.int32)
    xw = np.where(li == ri, np.float32(1.0), x_lerp).astype(np.float32)
    return ti, bi, y_lerp, xs, xw, valid_y, valid_x


def _build_core_inputs(boxes_k):
    """Per-core gather indices + per-slot weights for M_pad boxes."""
    m_pad = boxes_k.shape[0]
    assert m_pad % CH == 0
    nch = m_pad // CH
    ti, bi, yl, xs, xw, vy, vx = _grid_params(boxes_k)

    # per (box, i, j) flattened to q within each chunk
    b = np.arange(m_pad)
    top_desc = (ti[:, :, None] * W + xs[:, None, :]).reshape(m_pad, PX)
    bot_desc = (bi[:, :, None] * W + xs[:, None, :]).reshape(m_pad, PX)
    xw_q = np.broadcast_to(xw[:, None, :], (m_pad, CROP, CROP)).reshape(m_pad, PX)
    yw_q = np.broadcast_to(yl[:, :, None], (m_pad, CROP, CROP)).reshape(m_pad, PX)
    vm_q = (vy[:, :, None] & vx[:, None, :]).reshape(m_pad, PX).astype(np.float32)

    idx_all = np.zeros((nch, NI), np.int16)
    w_all = np.zeros((nch, P, S * 3), np.float32)
    for ch in range(nch):
        sl = slice(ch * CH, (ch + 1) * CH)
        t = top_desc[sl].reshape(-1)
        btm = bot_desc[sl].reshape(-1)
        descs = np.zeros(NI, np.int16)
        descs[: t.size] = t
        descs[QPAD : QPAD + btm.size] = btm
        idx_all[ch] = descs
        wq = np.zeros((3, QPAD), np.float32)
        wq[0, : t.size] = xw_q[sl].reshape(-1)
        wq[1, : t.size] = yw_q[sl].reshape(-1)
        wq[2, : t.size] = vm_q[sl].reshape(-1)
        # slot g, partition p <- q = g*128+p ; layout [P, S*3] = [p, g*3+c]
        wg = wq.reshape(3, S, P).transpose(2, 1, 0).reshape(P, S * 3)
        w_all[ch] = wg
    # wrapped idx layout: [16, NI//16] idx k at (k%16, k//16), tiled to 128
    wrapped = idx_all.reshape(nch, NI // 16, 16).transpose(0, 2, 1)
    idx_wrapped = np.tile(wrapped, (1, NCORES, 1))  # [nch, 128, NI//16]
    return idx_wrapped, w_all


def _build_program(nch):
    nc = bacc.Bacc("TRN2", target_bir_lowering=False, debug=False,
                   num_devices=NCORES)
    img = nc.dram_tensor("img", [NPIX * C], F32, kind="ExternalInput")
    idx = nc.dram_tensor("idx", [nch, P, NI // 16], I16, kind="ExternalInput")
    wts = nc.dram_tensor("wts", [nch, P, S * 3], F32, kind="ExternalInput")
    out = nc.dram_tensor("out", [nch * QPAD * C], F32, kind="ExternalOutput")

    # overlapping gather view: index unit = one 256-f32 column, payload = 2
    gather_src = bass.AP(img, 0, [(C, NPIX - 1), (1, 2 * C)])

    with tile.TileContext(nc) as tc:
        with (
            tc.tile_pool(name="gat", bufs=2) as gat_pool,
            tc.tile_pool(name="osb", bufs=2) as out_pool,
            tc.tile_pool(name="meta", bufs=2) as meta_pool,
            tc.tile_pool(name="tmp", bufs=4) as tmp_pool,
        ):
            nc.gpsimd.load_library(library_config.mlp)
            for ch in range(nch):
                idx_t = meta_pool.tile([P, NI // 16], I16, tag="idx")
                nc.sync.dma_start(idx_t[:], idx[ch])
                w_t = meta_pool.tile([P, S * 3], F32, tag="wts")
                nc.sync.dma_start(w_t[:], wts[ch])

                g = gat_pool.tile([P, 2 * S, 2 * C], F32, tag="g")
                # SWDGE ring tops out between 512 and 1664 descriptors per
                # instruction on this path; 512-desc sub-gathers are safe.
                GU = 512
                for j0 in range(0, NI, GU):
                    nj = min(GU, NI - j0)
                    nc.gpsimd.dma_gather(
                        g[:, j0 // P: (j0 + nj) // P, :], gather_src,
                        idx_t[:, j0 // 16: (j0 + nj) // 16], nj, nj,
                        2 * C, elem_step=C)

                o = out_pool.tile([P, S, C], F32, tag="o")
                for sgi in range(S):
                    t0 = g[:, sgi, 0:C]
                    t1 = g[:, sgi, C:2 * C]
                    b0 = g[:, S + sgi, 0:C]
                    b1 = g[:, S + sgi, C:2 * C]
                    xw_ap = w_t[:, sgi * 3 + 0: sgi * 3 + 1]
                    yw_ap = w_t[:, sgi * 3 + 1: sgi * 3 + 2]
                    vm_ap = w_t[:, sgi * 3 + 2: sgi * 3 + 3]

                    d_t = tmp_pool.tile([P, C], F32, tag="dt")
                    nc.gpsimd.tensor_tensor(d_t[:], t1, t0, SUB)
                    top = tmp_pool.tile([P, C], F32, tag="top")
                    nc.vector.scalar_tensor_tensor(top[:], d_t[:], xw_ap, t0,
                                                   MULT, ADD)
                    d_b = tmp_pool.tile([P, C], F32, tag="db")
                    nc.vector.tensor_tensor(d_b[:], b1, b0, SUB)
                    bot = tmp_pool.tile([P, C], F32, tag="bot")
                    nc.vector.scalar_tensor_tensor(bot[:], d_b[:], xw_ap, b0,
                                                   MULT, ADD)
                    d_v = tmp_pool.tile([P, C], F32, tag="dv")
                    nc.vector.tensor_tensor(d_v[:], bot[:], top[:], SUB)
                    val = tmp_pool.tile([P, C], F32, tag="val")
                    nc.vector.scalar_tensor_tensor(val[:], d_v[:], yw_ap,
                                                   top[:], MULT, ADD)
                    nc.scalar.mul(o[:, sgi, :], val[:], vm_ap)

                out_ap = bass.AP(out, ch * QPAD * C,
                                 [(C, P), (P * C, S), (1, C)])
                nc.scalar.dma_start(out_ap, o[:])

    nc.compile()
    return nc


def kernel(image, boxes, box_ind):
    image = np.asarray(image, dtype=np.float32)
    boxes = np.asarray(boxes, dtype=np.float32)
    box_ind = np.asarray(box_ind)
    n_boxes = boxes.shape[0]

    # group boxes by image; pad every core to a common multiple of CH
    sel = [np.where(box_ind == k)[0] for k in range(NCORES)]
    m_max = max(len(s) for s in sel)
    m_pad = ((m_max + CH - 1) // CH) * CH
    nch = m_pad // CH
    dummy = np.array([[0.25, 0.25, 0.75, 0.75]], np.float32)

    image_t = np.ascontiguousarray(image.transpose(0, 2, 3, 1))  # [B,H,W,C]

    in_maps = []
    for k in range(NCORES):
        bk = boxes[sel[k]]
        if bk.shape[0] < m_pad:
            bk = np.concatenate(
                [bk, np.repeat(dummy, m_pad - bk.shape[0], 0)], axis=0)
        idx_w, w_all = _build_core_inputs(bk)
        in_maps.append({
            "img": image_t[k].reshape(-1),
            "idx": idx_w,
            "wts": w_all,
        })

    key = nch
    if key not in _cache:
        _cache[key] = _build_program(nch)
    nc = _cache[key]

    res = bass_utils.run_bass_kernel_spmd(nc, in_maps,
                                          core_ids=list(range(NCORES)))
    global LAST_EXEC_NS
    LAST_EXEC_NS = res.exec_time_ns

    out = np.zeros((n_boxes, C, CROP, CROP), np.float32)
    for k in range(NCORES):
        ok = res.results[k]["out"].reshape(nch, QPAD, C)[:, : CH * PX, :]
        ok = ok.reshape(m_pad, PX, C)[: len(sel[k])]
        out[sel[k]] = ok.transpose(0, 2, 1).reshape(-1, C, CROP, CROP)
    return out



# revision 3
# speedup vs baseline: 7.8177x; 7.8177x over previous
"""CropAndResize (TF-style, crop 14x14) on 8 Trainium2 NeuronCores.

Strategy (data-parallel over ROIs, ~128 boxes per core):
  - Host: build a bf16 row-pair image per batch entry in channel-last
    layout: pairs[h, w] = (img[h, w, :], img[h+1, w, :]) -> [H-1, W, 2C].
    With this layout the 4 bilinear corners of one output pixel (rows
    ti/ti+1, cols xs/xs+1) are ONE contiguous 4C read (2 KB bf16), so a
    single SWDGE gather descriptor fetches a whole pixel's inputs.
  - Host: balance boxes across the 8 cores (each core = its own image
    plus at most one "secondary" donor image, concatenated in DRAM so
    int16 gather indices cover both), and precompute per-pixel corner
    weights w = [w_tl, w_bl, w_tr, w_br] * valid in f32.
  - Device: flat pipeline over groups of 512 pixels: one dma_gather
    (512 descriptors) -> per 128-pixel slot the weighted 4-corner sum
        out = w_tl*tl + w_bl*bl + w_tr*tr + w_br*br
    runs as 1 scalar-engine ACT + 3 vector STT ops (bf16 data, f32
    per-partition weights); results stream back to DRAM in bf16.
    GpSimd does nothing but descriptor generation, which is the
    critical engine (~4.7us per 512-desc gather).
  - Host: scatter per-core outputs back to original box order, upcast.
"""

import numpy as np
import ml_dtypes

import concourse.bacc as bacc
import concourse.bass as bass
import concourse.tile as tile
from concourse import mybir, library_config, bass_utils

H, W, C = 100, 152, 256
CROP = 14
PX = CROP * CROP          # 196 pixels per box
P = 128                   # SBUF partitions
NCORES = 8
NPIX2 = (H - 1) * W       # 15048 pair-columns per image
C2 = 2 * C                # elems per pair-column (bf16)
C4 = 4 * C                # gather element: 2 adjacent pair-columns
GU = 512                  # indices per dma_gather instruction
SG = GU // P              # slots per gather group (4)

F32 = mybir.dt.float32
BF16 = mybir.dt.bfloat16
I16 = mybir.dt.int16
MULT = mybir.AluOpType.mult
ADD = mybir.AluOpType.add

_cache = {}
LAST_EXEC_NS = None


def _grid_params(boxes):
    """f32 mirror of the reference sampling grid -> effective pair-row
    start ts, pair-col start xs, and the 4 corner weights (valid-masked)."""
    f = np.float32
    y1, x1, y2, x2 = boxes[:, 0], boxes[:, 1], boxes[:, 2], boxes[:, 3]
    h_scale = (y2 - y1) * f(H - 1) / f(CROP - 1)
    w_scale = (x2 - x1) * f(W - 1) / f(CROP - 1)
    ar = np.arange(CROP, dtype=np.float32)
    in_y = y1[:, None] * f(H - 1) + ar[None, :] * h_scale[:, None]
    in_x = x1[:, None] * f(W - 1) + ar[None, :] * w_scale[:, None]
    valid_y = (in_y >= 0) & (in_y <= H - 1)
    valid_x = (in_x >= 0) & (in_x <= W - 1)
    top = np.floor(in_y)
    left = np.floor(in_x)
    y_lerp = (in_y - top).astype(np.float32)
    x_lerp = (in_x - left).astype(np.float32)
    ti = np.clip(top, 0, H - 1).astype(np.int32)
    bi = np.clip(top + 1, 0, H - 1).astype(np.int32)
    li = np.clip(left, 0, W - 1).astype(np.int32)
    ri = np.clip(left + 1, 0, W - 1).astype(np.int32)
    # pair-row start + effective lerps (ti==bi -> bottom row of the pair;
    # li==ri -> right col of the pair; invalid pixels are masked anyway)
    ts = np.minimum(ti, H - 2).astype(np.int32)
    yw = np.where(ti == bi, np.float32(1.0), y_lerp).astype(np.float32)
    xs = np.minimum(li, W - 2).astype(np.int32)
    xw = np.where(li == ri, np.float32(1.0), x_lerp).astype(np.float32)

    n = boxes.shape[0]
    yw2 = yw[:, :, None]
    xw2 = xw[:, None, :]
    vm = (valid_y[:, :, None] & valid_x[:, None, :]).astype(np.float32)
    w4 = np.empty((n, CROP, CROP, 4), np.float32)
    w4[..., 0] = (1 - yw2) * (1 - xw2) * vm   # tl
    w4[..., 1] = yw2 * (1 - xw2) * vm         # bl
    w4[..., 2] = (1 - yw2) * xw2 * vm         # tr
    w4[..., 3] = yw2 * xw2 * vm               # br
    idx = (ts[:, :, None] * W + xs[:, None, :]).reshape(n, PX)
    return idx, w4.reshape(n, PX, 4)


def _assign(box_ind):
    """Balance boxes so each core hosts <= L boxes from <= 2 images
    (its own + one donor). Returns per-core global box-index lists,
    per-core secondary image id, and L."""
    n = len(box_ind)
    counts = np.bincount(box_ind, minlength=NCORES)
    lists = [list(np.where(box_ind == k)[0]) for k in range(NCORES)]
    assign = {}
    L = int(counts.max())
    for L in range(-(-n // NCORES), int(counts.max()) + 1):
        donors = {k: int(counts[k]) - L for k in range(NCORES) if counts[k] > L}
        rooms = {k: L - int(counts[k]) for k in range(NCORES) if counts[k] < L}
        trial = {}
        ok = True
        for d, shed in sorted(donors.items(), key=lambda x: -x[1]):
            rem = shed
            for r, room in sorted(rooms.items(), key=lambda x: -x[1]):
                if rem <= 0:
                    break
                take = min(room, rem)
                if take > 0:
                    trial[r] = (d, take)
                    rem -= take
            for r in trial:
                rooms.pop(r, None)
            if rem > 0:
                ok = False
                break
        if ok:
            assign = trial
            break
    pulled = {}
    by_donor = {}
    for r, (d, take) in assign.items():
        by_donor.setdefault(d, []).append((r, take))
    for d, rts in by_donor.items():
        tail = lists[d]
        pos = len(tail)
        for r, take in rts:
            pulled[r] = tail[pos - take: pos]
            pos -= take
        lists[d] = tail[:pos]
    core_boxes, core_sec = [], []
    for k in range(NCORES):
        extra = pulled.get(k, [])
        sec = int(box_ind[extra[0]]) if extra else k
        core_boxes.append(list(lists[k]) + list(extra))
        core_sec.append(sec)
    return core_boxes, core_sec, L


def _build_core_inputs(boxes_k, sel_k, ng):
    """Gather indices (wrapped int16) + per-slot weights for one core.
    boxes_k: [m, 4]; sel_k: [m] in {0,1} (primary/secondary image)."""
    ntot = ng * GU
    m = boxes_k.shape[0]
    idx, w4 = _grid_params(boxes_k)                      # [m,196], [m,196,4]
    idx = idx + (np.asarray(sel_k, np.int64)[:, None] * NPIX2)
    idx_flat = np.zeros(ntot, np.int16)
    idx_flat[: m * PX] = idx.reshape(-1).astype(np.int16)
    w_flat = np.zeros((ntot, 4), np.float32)
    w_flat[: m * PX] = w4.reshape(-1, 4)
    # wrapped idx layout: idx k at (k%16, k//16), tiled to 128 partitions
    wrapped = idx_flat.reshape(ntot // 16, 16).T         # [16, ntot//16]
    idx_w = np.tile(wrapped, (P // 16, 1))               # [128, ntot//16]
    # weights: [p, slot*4 + c] = w_flat[slot*128 + p, c]
    w_all = np.ascontiguousarray(
        w_flat.reshape(ntot // P, P, 4).transpose(1, 0, 2).reshape(P, -1))
    return idx_w, w_all


def _build_program(ng):
    nc = bacc.Bacc("TRN2", target_bir_lowering=False, debug=False,
                   num_devices=NCORES)
    img = nc.dram_tensor("img", [2 * NPIX2 * C2], BF16, kind="ExternalInput")
    idx = nc.dram_tensor("idx", [P, ng * (GU // 16)], I16, kind="ExternalInput")
    wts = nc.dram_tensor("wts", [P, ng * SG * 4], F32, kind="ExternalInput")
    out = nc.dram_tensor("out", [ng * GU * C], BF16, kind="ExternalOutput")

    # overlapping gather view: index unit = one pair-column (C2 bf16),
    # payload = 2 adjacent pair-columns (C4 bf16 = 2 KB)
    gather_src = bass.AP(img, 0, [(C2, 2 * NPIX2 - 1), (1, C4)])

    with tile.TileContext(nc) as tc:
        with (
            tc.tile_pool(name="gat", bufs=8) as gat_pool,
            tc.tile_pool(name="osb", bufs=4) as out_pool,
            tc.tile_pool(name="meta", bufs=1) as meta_pool,
            tc.tile_pool(name="tmp", bufs=4) as tmp_pool,
        ):
            nc.gpsimd.load_library(library_config.mlp)
            idx_t = meta_pool.tile([P, ng * (GU // 16)], I16, tag="idx")
            nc.sync.dma_start(idx_t[:], idx[:])
            w_t = meta_pool.tile([P, ng * SG * 4], F32, tag="wts")
            nc.sync.dma_start(w_t[:], wts[:])

            for g in range(ng):
                gt = gat_pool.tile([P, SG, C4], BF16, tag="g")
                nc.gpsimd.dma_gather(
                    gt[:], gather_src,
                    idx_t[:, g * (GU // 16): (g + 1) * (GU // 16)],
                    GU, GU, C4, elem_step=C2)
                ot = out_pool.tile([P, SG, C], BF16, tag="o")
                for s in range(SG):
                    k = (g * SG + s) * 4
                    tl = gt[:, s, 0:C]
                    bl = gt[:, s, C:2 * C]
                    tr = gt[:, s, 2 * C:3 * C]
                    br = gt[:, s, 3 * C:4 * C]
                    a0 = tmp_pool.tile([P, C], BF16, tag="a0")
                    nc.scalar.mul(a0[:], tl, w_t[:, k: k + 1])
                    a1 = tmp_pool.tile([P, C], BF16, tag="a1")
                    nc.vector.scalar_tensor_tensor(
                        a1[:], bl, w_t[:, k + 1: k + 2], a0[:], MULT, ADD)
                    a2 = tmp_pool.tile([P, C], BF16, tag="a2")
                    nc.vector.scalar_tensor_tensor(
                        a2[:], tr, w_t[:, k + 2: k + 3], a1[:], MULT, ADD)
                    nc.vector.scalar_tensor_tensor(
                        ot[:, s, :], br, w_t[:, k + 3: k + 4], a2[:],
                        MULT, ADD)
                out_ap = bass.AP(out, g * GU * C, [(C, P), (P * C, SG), (1, C)])
                nc.sync.dma_start(out_ap, ot[:])

    nc.compile()
    return nc


def kernel(image, boxes, box_ind):
    image = np.asarray(image, dtype=np.float32)
    boxes = np.asarray(boxes, dtype=np.float32)
    box_ind = np.asarray(box_ind)
    n_boxes = boxes.shape[0]

    core_boxes, core_sec, m_pad = _assign(box_ind)
    ng = -(-(m_pad * PX) // GU)

    # channel-last bf16 row-pair images: pairs[k][h*W+w] = img[h,w,:]+img[h+1,w,:]
    image_t = image.transpose(0, 2, 3, 1).astype(ml_dtypes.bfloat16)  # [B,H,W,C]
    pairs = np.concatenate([image_t[:, :-1], image_t[:, 1:]], axis=-1)
    pairs = pairs.reshape(NCORES, NPIX2 * C2)

    dummy = np.array([[0.25, 0.25, 0.75, 0.75]], np.float32)
    in_maps = []
    for k in range(NCORES):
        gsel = core_boxes[k]
        bk = boxes[gsel]
        sel_k = (box_ind[gsel] != k).astype(np.int64)
        npad = m_pad - bk.shape[0]
        if npad:
            bk = np.concatenate([bk, np.repeat(dummy, npad, 0)], axis=0)
            sel_k = np.concatenate([sel_k, np.zeros(npad, np.int64)])
        idx_w, w_all = _build_core_inputs(bk, sel_k, ng)
        in_maps.append({
            "img": np.concatenate([pairs[k], pairs[core_sec[k]]]),
            "idx": idx_w,
            "wts": w_all,
        })

    if ng not in _cache:
        _cache[ng] = _build_program(ng)
    nc = _cache[ng]

    res = bass_utils.run_bass_kernel_spmd(nc, in_maps,
                                          core_ids=list(range(NCORES)))
    global LAST_EXEC_NS
    LAST_EXEC_NS = res.exec_time_ns

    out = np.zeros((n_boxes, C, CROP, CROP), np.float32)
    for k in range(NCORES):
        nb = len(core_boxes[k])
        ok = np.asarray(res.results[k]["out"]).reshape(-1, C)[: nb * PX]
        ok = ok.reshape(nb, PX, C).transpose(0, 2, 1).astype(np.float32)
        out[core_boxes[k]] = ok.reshape(nb, C, CROP, CROP)
    return out
